# revision 19
# baseline (speedup 1.0000x reference)
"""ComplexLSTM Trainium2 kernel.

Problem: x [2, 64, 128, 1024] (real/imag, B, I, T) -> out [2, 64, 256, 1024].
Four real LSTM applications combined as L_r = r(xr) - i(xim),
L_i = r(xim) + i(xr).

Sharding (output-parallel, combine on device): 8 cores x 16 output rows.
  core j<4:  out[0, 16j:16j+16] = lstm_r(x_real[bj]) - lstm_i(x_imag[bj])
  core j>=4: out[1, 16(j-4)..]  = lstm_r(x_imag[bj]) + lstm_i(x_real[bj])
Every core runs TWO interleaved LSTM chains (A: r-weights, B: i-weights,
batch 16 each); their step tails overlap with the other chain's matmuls.
The +- sign enters via a per-core constant tile, so the program is SPMD.

Device layout per chain (transposed state, weights-stationary matmuls):
  gates.T accumulated in PSUM as [128p, 8 blocks, 16 batch]; block j = gate
  rows 128j..127, gate order [g,g,i,i,f,f,o,o]. bias via indicator matmul,
  x-proj via 8 MMs (lhsT=WihT tiles), recurrence via 16 MMs (lhsT=WhhT
  tiles bf16 FWL, rhs=h.T slice of the history buffer).

I/O keeps the host idle: inputs are bf16 slices of x (one vectorized RNE
cast on host); the combined output leaves the device already in the final
[b, H, T] layout as bf16; host does one fused upcast into the result.
"""

import numpy as np
import ml_dtypes
from contextlib import ExitStack

import concourse.bass as bass
import concourse.bacc as bacc
import concourse.tile as tile
from concourse import mybir
from concourse.bass_utils import run_bass_kernel_spmd

BF16 = mybir.dt.bfloat16
F32 = mybir.dt.float32
AF = mybir.ActivationFunctionType
OP = mybir.AluOpType

B, I, T_FULL, H = 64, 128, 1024, 256
NB = 16          # batch (sequences) per chain; 2 chains per core
NCORES = 8
CH = 128         # T chunk (steps per input/output DMA)

_cache = {}


def build(T):
    nc = bacc.Bacc("TRN2", target_bir_lowering=False, debug=False)

    ch = max(1, min(CH, T))
    assert T % ch == 0

    xa_d = nc.declare_dram_parameter("xa", [NB, 128, T], BF16, isOutput=False)
    xb_d = nc.declare_dram_parameter("xb", [NB, 128, T], BF16, isOutput=False)
    whhT_d = nc.declare_dram_parameter("whhT", [128, 2, 2, 8, 128], BF16, isOutput=False)
    wihT_d = nc.declare_dram_parameter("wihT", [128, 2, 8, 128], BF16, isOutput=False)
    biasK_d = nc.declare_dram_parameter("biasK", [8, 2, 128], BF16, isOutput=False)
    ind_d = nc.declare_dram_parameter("ind", [8, 8 * NB], BF16, isOutput=False)
    sgn_d = nc.declare_dram_parameter("sgn", [128, 1], BF16, isOutput=False)
    out_d = nc.declare_dram_parameter("out", [NB, 2, 128, T], BF16, isOutput=True)

    with tile.TileContext(nc) as tc, ExitStack() as ctx:
        consts = ctx.enter_context(tc.tile_pool(name="consts", bufs=1))
        xraw = ctx.enter_context(tc.tile_pool(name="xraw", bufs=2))
        xin = ctx.enter_context(tc.tile_pool(name="xin", bufs=2))
        hpool = ctx.enter_context(tc.tile_pool(name="hist", bufs=2))
        lpool = ctx.enter_context(tc.tile_pool(name="lout", bufs=2))
        psum = ctx.enter_context(tc.tile_pool(name="psum", bufs=2, space="PSUM"))
        sml = ctx.enter_context(tc.tile_pool(name="small", bufs=6))
        cpool = ctx.enter_context(tc.tile_pool(name="cpool", bufs=6))

        WHH = consts.tile([128, 2, 2, 8, 128], BF16)
        nc.sync.dma_start(WHH[:], whhT_d[:])
        WIH = consts.tile([128, 2, 8, 128], BF16)
        nc.sync.dma_start(WIH[:], wihT_d[:])
        BIASK = consts.tile([8, 2, 128], BF16)
        nc.sync.dma_start(BIASK[:], biasK_d[:])
        IND = consts.tile([8, 8 * NB], BF16)
        nc.sync.dma_start(IND[:], ind_d[:])
        SGN = consts.tile([128, 1], BF16)
        nc.sync.dma_start(SGN[:], sgn_d[:])

        xT = [xa_d[:].transpose([1, 0, 2]), xb_d[:].transpose([1, 0, 2])]

        XT = [None, None]
        HIST = [None, None]
        LOUT = None
        c_prev = [None, None]
        h_prev = [None, None]
        h_cur = [None, None]

        for t in range(T):
            tl = t % ch
            if tl == 0:
                for a in range(2):
                    XR = xraw.tile([128, NB, ch], BF16, tag=f"xr{a}")
                    nc.sync.dma_start(XR[:], xT[a][:, :, t:t + ch])
                    XT[a] = xin.tile([128, ch, NB], BF16, tag=f"xt{a}", name=f"XT{a}")
                    nc.vector.tensor_copy(XT[a][:], XR[:].transpose([0, 2, 1]))
                    HIST[a] = hpool.tile([128, 2, NB, ch], BF16, tag=f"hist{a}", name=f"HIST{a}")
                LOUT = lpool.tile([128, 2, NB, ch], BF16, tag="lout")

            for a in range(2):
                g_ps = psum.tile([128, 8, NB], F32, tag=f"gates{a}")
                nc.tensor.matmul(g_ps[:], BIASK[:, a, :], IND[:], start=True, stop=False)
                for m in range(8):
                    nc.tensor.matmul(
                        g_ps[:, m, :], WIH[:, a, m, :], XT[a][:, tl, :],
                        start=False, stop=False,
                    )
                if t > 0:
                    for m in range(8):
                        for k in range(2):
                            nc.tensor.matmul(
                                g_ps[:, m, :], WHH[:, a, k, m, :], h_prev[a][:, k, :],
                                start=False, stop=(k == 1),
                            )

                # activations: blocks [0:2]=g (tanh), [2:8]=i,f,o (sigmoid)
                sg = sml.tile([128, 6, NB], F32, tag=f"sg{a}")
                nc.scalar.activation(sg[:], g_ps[:, 2:8, :], AF.Sigmoid)
                gt = sml.tile([128, 2, NB], F32, tag=f"gt{a}")
                nc.scalar.activation(gt[:], g_ps[:, 0:2, :], AF.Tanh)

                v = sml.tile([128, 2, NB], F32, tag=f"v{a}")
                nc.vector.tensor_tensor(v[:], sg[:, 0:2, :], gt[:], OP.mult)
                c_new = cpool.tile([128, 2, NB], F32, tag=f"c{a}")
                if t > 0:
                    u = sml.tile([128, 2, NB], F32, tag=f"u{a}")
                    nc.vector.tensor_tensor(u[:], sg[:, 2:4, :], c_prev[a][:], OP.mult)
                    nc.vector.tensor_tensor(c_new[:], u[:], v[:], OP.add)
                else:
                    nc.vector.tensor_copy(c_new[:], v[:])
                tch = sml.tile([128, 2, NB], F32, tag=f"tch{a}")
                nc.scalar.activation(tch[:], c_new[:], AF.Tanh)
                h_slot = HIST[a][:, :, :, tl]
                nc.vector.tensor_tensor(h_slot, sg[:, 4:6, :], tch[:], OP.mult)

                c_prev[a] = c_new
                h_cur[a] = h_slot
                h_prev[a] = h_slot

            # combine: L = hA + sgn * hB  (sgn is +-1 per core)
            hb_s = sml.tile([128, 2, NB], F32, tag="hbs")
            nc.vector.tensor_tensor(hb_s[:], h_cur[1],
                                    SGN[:].broadcast_to([128, 2, NB]),
                                    OP.mult)
            nc.vector.tensor_tensor(LOUT[:, :, :, tl], h_cur[0], hb_s[:], OP.add)

            if tl == ch - 1:
                t0 = t - (ch - 1)
                for hc in range(2):
                    nc.sync.dma_start(
                        out_d[:, hc, :, t0:t0 + ch].transpose([1, 0, 2]),
                        LOUT[:, hc, :, :])
    nc.compile()
    return nc


def _get_nc(T):
    if T not in _cache:
        _cache[T] = build(T)
    return _cache[T]


def _prep_weights(Wih, Whh, bih, bhh):
    """Per weight-set host prep of the (small) weight tensors."""
    # gate permutation torch [i,f,g,o] -> [g,i,f,o]
    perm = np.concatenate([np.arange(512, 768), np.arange(0, 256),
                           np.arange(256, 512), np.arange(768, 1024)])
    Wihp = np.asarray(Wih)[perm]          # [1024, 128]
    Whhp = np.asarray(Whh)[perm]          # [1024, 256]
    biasp = (np.asarray(bih) + np.asarray(bhh))[perm]  # [1024]

    whhT = Whhp.reshape(8, 128, 2, 128).transpose(3, 2, 0, 1)  # [p,k,m,j]
    wihT = Wihp.reshape(8, 128, 128).transpose(2, 0, 1)        # [p,m,j]
    biasK = biasp.reshape(8, 128)
    whhT = np.ascontiguousarray(whhT).astype(ml_dtypes.bfloat16)
    wihT = np.ascontiguousarray(wihT).astype(ml_dtypes.bfloat16)
    biasK = biasK.astype(ml_dtypes.bfloat16)
    return whhT, wihT, biasK


def _bf16_rne(a):
    """float32 -> bfloat16 with round-to-nearest-even, vectorized."""
    u = np.asarray(a, np.float32).view(np.uint32)
    r = ((u + 0x7FFF + ((u >> 16) & 1)) >> 16).astype(np.uint16)
    return r.view(ml_dtypes.bfloat16)


def _run(x, Wih_r, Whh_r, bih_r, bhh_r, Wih_i, Whh_i, bih_i, bhh_i, T,
         trace=False, tmpdir=None):
    nc = _get_nc(T)
    ind = np.kron(np.eye(8), np.ones((1, NB))).astype(ml_dtypes.bfloat16)

    w_r = _prep_weights(Wih_r, Whh_r, bih_r, bhh_r)
    w_i = _prep_weights(Wih_i, Whh_i, bih_i, bhh_i)
    # stack weight sets: dim order matches whhT_d [128, set, k, m, j] etc.
    whhT = np.ascontiguousarray(np.stack([w_r[0], w_i[0]], axis=1))
    wihT = np.ascontiguousarray(np.stack([w_r[1], w_i[1]], axis=1))
    biasK = np.ascontiguousarray(np.stack([w_r[2], w_i[2]], axis=1))

    xb16 = _bf16_rne(x)  # [2, B, 128, T] bf16, one pass
    sgn_m = np.full((128, 1), -1.0, ml_dtypes.bfloat16)
    sgn_p = np.full((128, 1), 1.0, ml_dtypes.bfloat16)

    in_maps = []
    for core in range(NCORES):
        half = core // 4          # 0: L_r rows, 1: L_i rows
        j = core % 4
        b0 = 16 * j
        if half == 0:
            xa, xb = xb16[0, b0:b0 + 16], xb16[1, b0:b0 + 16]
            sgn = sgn_m
        else:
            xa, xb = xb16[1, b0:b0 + 16], xb16[0, b0:b0 + 16]
            sgn = sgn_p
        in_maps.append({
            "xa": xa, "xb": xb, "whhT": whhT, "wihT": wihT,
            "biasK": biasK, "ind": ind, "sgn": sgn,
        })
    res = run_bass_kernel_spmd(nc, in_maps, core_ids=list(range(NCORES)),
                               trace=trace, tmpdir=tmpdir)
    results = res.results

    out = np.empty((2, B, 2 * 128, T), np.float32)
    for core in range(NCORES):
        half, j = core // 4, core % 4
        dst = out[half, 16 * j:16 * j + 16]  # [16, 256, T] contiguous
        src = results[core]["out"].reshape(NB, 2 * 128, T)
        np.left_shift(src.view(np.uint16).astype(np.uint32), 16,
                      out=dst.view(np.uint32))
    return out, res


def kernel(x, Wih_r, Whh_r, bih_r, bhh_r, Wih_i, Whh_i, bih_i, bhh_i):
    out, _ = _run(x, Wih_r, Whh_r, bih_r, bhh_r,
                  Wih_i, Whh_i, bih_i, bhh_i, T_FULL)
    return out
